# revision 31
# baseline (speedup 1.0000x reference)
"""Causal self-attention Trainium2 kernel (8-core SPMD).

Problem: x[2,2048,1024], causal mask, Wqkv[3072,1024], Wo[1024,1024], fp32.
  qkv = x @ Wqkv.T ; per-head causal softmax attention ; out = attn @ Wo.T

Sharding (data + tensor parallel over heads):
  core c -> batch b = c // 4, heads {4g..4g+3} with g = c % 4.
  Each core computes Q,K,V for its 4 heads, runs causal attention for them,
  and multiplies by the matching 256 columns of Wo, producing a partial
  [2048, 1024] bf16 output. Host sums the 4 partials per batch in fp32.

Kernel structure (per core): baseline dense-emission skeleton with
  - bf16 matmul operands, fp32 PSUM accumulation.
  - PAIRED scores: heads 2p/2p+1 live at partition bases 0/64 of the same
    qkT m-tile, so their K^T Q matmuls (64-row stationaries) execute
    CONCURRENTLY in opposite PE row groups (row tiling), writing the two
    banks of one [128,1024] PSUM tile; one 1024-wide exp covers both.
  - AV per head unsplit ([128,65] stationary with a ones column accumulating
    the softmax denominator), emitted 2 k-blocks behind the paired scores so
    the PE tracks ACT's exp rate; normalization reads PSUM directly.
  - Causality: strictly-upper blocks skipped; diagonal straddlers compute
    only valid columns; the 128-wide diagonal block is exp'd unmasked and
    both heads are masked with one strided multiply against a stacked
    binary mask tile.
  - Input DMAs chunked [128,512] over the 3 DMA queues in consumption
    order, so the first projection chain starts after ~300KB.
"""

import os

import numpy as np

S = 2048
D = 1024
DH = 64
B = 2
NCORES = 8
HPC = 4  # heads per core
QKC = 2 * HPC * DH  # 512 q+k projection columns per core
VC = HPC * DH  # 256 v columns per core
P = 128
H = 64
KO = D // P  # 8 contraction tiles
NQ = S // 512  # 4 q-chunks of 512
NSC = S // P  # 16 s-chunks of 128

COMPUTE_DT = os.environ.get("ATTN_COMPUTE_DT", "bf16")
DEBUG = os.environ.get("ATTN_DEBUG", "0") == "1"

_cache = {}


def _np_compute_dt():
    if COMPUTE_DT == "bf16":
        import ml_dtypes

        return ml_dtypes.bfloat16
    return np.float32


def _build():
    import concourse.bacc as bacc
    import concourse.mybir as mybir
    import concourse.tile as tile

    F32 = mybir.dt.float32
    CDT = mybir.dt.bfloat16 if COMPUTE_DT == "bf16" else mybir.dt.float32r
    EXP = mybir.ActivationFunctionType.Exp

    nc = bacc.Bacc()
    xT_d = nc.dram_tensor("xT", [D, S], CDT, kind="ExternalInput")
    wqkT_d = nc.dram_tensor("wqkT", [D, QKC], CDT, kind="ExternalInput")
    wvT_d = nc.dram_tensor("wvT", [D, VC], CDT, kind="ExternalInput")
    woT_d = nc.dram_tensor("woT", [VC, D], CDT, kind="ExternalInput")
    maskT_d = nc.dram_tensor("maskT", [P, P], CDT, kind="ExternalInput")
    out_d = nc.dram_tensor("out", [S, D], CDT, kind="ExternalOutput")
    if DEBUG:
        qkT_dump = nc.dram_tensor("qkT_dump", [P, 4, S], CDT, kind="ExternalOutput")
        v_dump = nc.dram_tensor("v_dump", [P, NSC, HPC, DH + 1], CDT, kind="ExternalOutput")
        attn_dump = nc.dram_tensor("attn_dump", [P, 2, S], CDT, kind="ExternalOutput")

    with tile.TileContext(nc) as tc:
        with (
            tc.tile_pool(name="persist", bufs=1) as persist,
            tc.tile_pool(name="sb_small", bufs=3) as sb_small,
            tc.tile_pool(name="sb_exp", bufs=12) as sb_exp,
            tc.tile_pool(name="sb_out", bufs=3) as sb_out,
            tc.tile_pool(name="pp_big", bufs=2, space="PSUM") as pp_big,
            tc.tile_pool(name="pp_av", bufs=2, space="PSUM") as pp_av,
            tc.tile_pool(name="pp_o", bufs=2, space="PSUM") as pp_o,
        ):
            xT_sb = persist.tile([P, KO, S], CDT, tag="xT")
            wqkT_sb = persist.tile([P, KO, QKC], CDT, tag="wqkT")
            wvT_sb = persist.tile([P, KO, VC], CDT, tag="wvT")
            woT_sb = persist.tile([P, 2, D], CDT, tag="woT")
            maskT2_sb = persist.tile([P, 2, P], CDT, tag="maskT2")
            qkT_sb = persist.tile([P, 4, S], CDT, tag="qkT")
            v_sb = persist.tile([P, NSC, HPC, DH + 1], CDT, tag="v")
            attn_sb = persist.tile([P, 2, S], CDT, tag="attn")

            def xdma(eng, ko, qc):
                eng.dma_start(
                    xT_sb[:, ko, qc * 512 : (qc + 1) * 512],
                    xT_d[ko * P : (ko + 1) * P, qc * 512 : (qc + 1) * 512],
                )

            # Round-robin all transfers across the 3 DMA queues in strict
            # consumption order: qk-proj inputs first (wqkT[ko] + x[ko,qc0]
            # pairs), then wvT + mask, then later x chunks, then woT.
            qs = [nc.sync, nc.gpsimd, nc.scalar]
            qi = 0

            def rr():
                nonlocal qi
                q = qs[qi % 3]
                qi += 1
                return q

            for ko in range(KO):
                rr().dma_start(wqkT_sb[:, ko, :], wqkT_d[ko * P : (ko + 1) * P, :])
                xdma(rr(), ko, 0)
            for ko in range(KO):
                rr().dma_start(wvT_sb[:, ko, :], wvT_d[ko * P : (ko + 1) * P, :])
            for _mh in range(2):
                rr().dma_start(maskT2_sb[:, _mh, :], maskT_d[:])
            for qc in range(1, NQ):
                for ko in range(KO):
                    xdma(rr(), ko, qc)
            rr().dma_start(woT_sb[:], woT_d.rearrange("(ct p) e -> p ct e", p=P))
            ones_f32 = persist.tile([P, DH], F32, tag="ones_f32")
            nc.vector.memset(ones_f32[:], 1.0)
            nc.vector.tensor_copy(
                out=v_sb[:, :, :, DH],
                in_=ones_f32[:, 0 : NSC * HPC].rearrange("p (a b) -> p a b", a=NSC),
            )

            def outproj_unit(qc, si, en):
                sc = qc * 4 + si
                ps_o = pp_o.tile([P, 512], F32, tag="o")
                for ct in range(2):
                    nc.tensor.matmul(
                        ps_o[:],
                        attn_sb[:, ct, sc * P : (sc + 1) * P],
                        woT_sb[:, ct, en * 512 : (en + 1) * 512],
                        start=(ct == 0),
                        stop=(ct == 1),
                        skip_group_check=True,
                    )
                o_sb = sb_out.tile([P, 512], CDT, tag="osb")
                nc.vector.tensor_copy(out=o_sb[:], in_=ps_o[:])
                (nc.sync if (si + en) % 2 == 0 else nc.gpsimd).dma_start(
                    out_d[sc * P : (sc + 1) * P, en * 512 : (en + 1) * 512],
                    o_sb[:],
                )

            def emit_outproj(qc):
                for si in range(4):
                    for en in range(2):
                        outproj_unit(qc, si, en)

            for qc in range(NQ):
                # --- qk projection: ko-outer over two 2-bank tiles (4
                # half-bank chains) so the PE tracks DMA arrival ---
                pjA = pp_big.tile([P, 1024], F32, tag="big", name="pjA")
                pjB = pp_big.tile([P, 1024], F32, tag="big", name="pjB")
                for ko in range(KO):
                    for mm in range(4):
                        slot = pjA if mm < 2 else pjB
                        nc.tensor.matmul(
                            slot[:, (mm % 2) * 512 : (mm % 2 + 1) * 512],
                            wqkT_sb[:, ko, mm * P : (mm + 1) * P],
                            xT_sb[:, ko, qc * 512 : (qc + 1) * 512],
                            start=(ko == 0),
                            stop=(ko == KO - 1),
                            skip_group_check=True,
                        )
                nc.vector.tensor_copy(
                    out=qkT_sb[:, 0:2, qc * 512 : (qc + 1) * 512],
                    in_=pjA.rearrange("p (a b) -> p a b", a=2),
                )
                nc.vector.tensor_copy(
                    out=qkT_sb[:, 2:4, qc * 512 : (qc + 1) * 512],
                    in_=pjB.rearrange("p (a b) -> p a b", a=2),
                )

                # --- v projection for s-chunks 4qc..4qc+3 (4 bank chains) ---
                pvA = pp_big.tile([P, 1024], F32, tag="big", name="pvA")
                pvB = pp_big.tile([P, 1024], F32, tag="big", name="pvB")
                for ko in range(KO):
                    for j in range(4):
                        slot = pvA if j < 2 else pvB
                        sc = 4 * qc + j
                        nc.tensor.matmul(
                            slot[:, (j % 2) * 512 : (j % 2) * 512 + VC],
                            xT_sb[:, ko, sc * P : (sc + 1) * P],
                            wvT_sb[:, ko, :],
                            start=(ko == 0),
                            stop=(ko == KO - 1),
                            skip_group_check=True,
                        )
                for half, slot in ((0, pvA), (1, pvB)):
                    nc.vector.tensor_copy(
                        out=v_sb[:, 4 * qc + 2 * half : 4 * qc + 2 * half + 2, :, 0:DH],
                        in_=slot.rearrange("p (a h d) -> p a h d", a=2, h=8)[:, :, 0:HPC, :],
                    )

                # --- attention for q-chunk qc, head pairs (0,1), (2,3).
                # Pair heads sit at partition bases 0/64 of one qkT m-tile,
                # so the two 64-row score matmuls of a k-block execute
                # concurrently in opposite PE row groups (one 2-bank tile,
                # one 1024-wide exp). AV (unsplit, ones-column denominator)
                # trails the scores by 2 k-blocks.
                nkb = 4 * qc + 4
                # out-projection units of the previous chunk are sprinkled
                # into the attention slot stream (own PSUM pool, so no ring
                # interaction): they fill the PE idle the ACT-paced scores
                # ring would otherwise leave, keeping HAM warm.
                units = (
                    [(qc - 1, si, en) for si in range(4) for en in range(2)]
                    if qc > 0
                    else []
                )
                ustep = max(1, (2 * nkb) // 8)
                slot_i = 0
                for pr in range(2):
                    mq = pr  # Q m-tile; K m-tile = 2 + pr
                    ps_avs = [
                        pp_av.tile([DH + 1, 512], F32, tag="av", name=f"av{hh}")
                        for hh in range(2)
                    ]
                    exps = {}

                    def emit_scores(kb):
                        m = kb - 4 * qc
                        off = max(0, m) * P
                        ps2 = pp_big.tile([P, 1024], F32, tag="big", name="ps2")
                        exp2 = sb_exp.tile([P, 1024], CDT, tag="exp")
                        for hh in range(2):
                            hp = hh * H
                            nc.tensor.matmul(
                                ps2[:, hh * 512 + off : hh * 512 + 512],
                                qkT_sb[hp : hp + H, 2 + mq, kb * P : (kb + 1) * P],
                                qkT_sb[hp : hp + H, mq, qc * 512 + off : (qc + 1) * 512],
                                start=True,
                                stop=True,
                                skip_group_check=True,
                            )
                        if off == 0:
                            nc.scalar.activation(exp2[:], ps2[:], EXP, scale=0.125)
                        else:
                            for hh in range(2):
                                lo = hh * 512 + off
                                nc.scalar.activation(
                                    exp2[:, lo : hh * 512 + 512],
                                    ps2[:, lo : hh * 512 + 512],
                                    EXP,
                                    scale=0.125,
                                )
                        if m >= 0:
                            e2v = exp2.rearrange("p (h q) -> p h q", h=2)[
                                :, :, off : off + P
                            ]
                            nc.vector.tensor_mul(out=e2v, in0=e2v, in1=maskT2_sb[:])
                        exps[kb] = (exp2, off)

                    def emit_av(kb):
                        exp2, off = exps[kb]
                        for hh in range(2):
                            h = 2 * pr + hh
                            lo = hh * 512 + off
                            nc.tensor.matmul(
                                ps_avs[hh][:, off:512],
                                v_sb[:, kb, h, :],
                                exp2[:, lo : (lo - off) + 512],
                                start=(kb == 0),
                                stop=(kb == nkb - 1),
                                skip_group_check=True,
                            )

                    for kb in range(nkb):
                        emit_scores(kb)
                        if kb >= 2:
                            emit_av(kb - 2)
                        slot_i += 1
                        if units and kb >= 2 and slot_i % ustep == 0:
                            outproj_unit(*units.pop(0))
                    emit_av(nkb - 2)
                    emit_av(nkb - 1)

                    # normalize: attn = av * (1/sums), reciprocal broadcast
                    # over the 64 head dims via GPSIMD (keeps the PE out of
                    # the pair-boundary dependency chain).
                    sums2 = sb_small.tile([1, 1024], F32, tag="sums2")
                    for hh in range(2):
                        nc.vector.tensor_copy(
                            out=sums2[:, hh * 512 : (hh + 1) * 512],
                            in_=ps_avs[hh][DH : DH + 1, :],
                        )
                    recip2 = sb_small.tile([1, 1024], F32, tag="recip2")
                    nc.vector.reciprocal_approx_fast(out=recip2[:], in_=sums2[:])
                    for hh in range(2):
                        bc_sb = sb_small.tile([DH, 512], F32, tag=f"bc{hh}")
                        nc.gpsimd.partition_broadcast(
                            bc_sb[:], recip2[:, hh * 512 : (hh + 1) * 512]
                        )
                        hp = hh * H
                        nc.vector.tensor_mul(
                            out=attn_sb[hp : hp + DH, pr, qc * 512 : (qc + 1) * 512],
                            in0=ps_avs[hh][0:DH, :],
                            in1=bc_sb[:],
                        )

                # any remaining deferred out-projection units
                while units:
                    outproj_unit(*units.pop(0))
            emit_outproj(NQ - 1)
            if DEBUG:
                nc.sync.dma_start(qkT_dump[:], qkT_sb[:])
                nc.sync.dma_start(v_dump[:], v_sb[:])
                nc.sync.dma_start(attn_dump[:], attn_sb[:])

    nc.compile()
    return nc


def _get_nc():
    if "nc" not in _cache:
        _cache["nc"] = _build()
    return _cache["nc"]


def _shard(x, mask, Wqkv, Wo):
    cdt = _np_compute_dt()
    in_maps = []
    # binary mask for the transposed 128x128 diagonal block:
    # valid (mask==0) -> 1.0, masked (-inf/large-negative) -> 0.0
    maskT = np.ascontiguousarray((mask[0, 0, :P, :P].T >= 0).astype(cdt))
    for c in range(NCORES):
        b = c // 4
        g = c % 4
        heads = [4 * g + i for i in range(HPC)]
        q_rows = np.concatenate([np.arange(h * DH, (h + 1) * DH) for h in heads])
        k_rows = D + q_rows
        v_rows = 2 * D + q_rows
        qk_rows = np.concatenate([q_rows, k_rows])
        in_maps.append(
            {
                "xT": np.ascontiguousarray(x[b].T.astype(cdt)),
                "wqkT": np.ascontiguousarray(Wqkv[qk_rows, :].T.astype(cdt)),
                "wvT": np.ascontiguousarray(Wqkv[v_rows, :].T.astype(cdt)),
                "woT": np.ascontiguousarray(Wo[:, q_rows].T.astype(cdt)),
                "maskT": maskT,
            }
        )
    return in_maps


def kernel(x, mask, Wqkv, Wo, _trace=False):
    from concourse.bass_utils import run_bass_kernel_spmd

    x = np.asarray(x, dtype=np.float32)
    mask = np.asarray(mask, dtype=np.float32)
    Wqkv = np.asarray(Wqkv, dtype=np.float32)
    Wo = np.asarray(Wo, dtype=np.float32)

    nc = _get_nc()
    in_maps = _shard(x, mask, Wqkv, Wo)
    res = run_bass_kernel_spmd(nc, in_maps, core_ids=list(range(NCORES)), trace=_trace)
    _cache["last_result"] = res

    out = np.zeros((B, S, D), dtype=np.float32)
    for c in range(NCORES):
        out[c // 4] += res.results[c]["out"].astype(np.float32)
    return out


# revision 32
# speedup vs baseline: 1.0121x; 1.0121x over previous
"""Causal self-attention Trainium2 kernel (8-core SPMD).

Problem: x[2,2048,1024], causal mask, Wqkv[3072,1024], Wo[1024,1024], fp32.
  qkv = x @ Wqkv.T ; per-head causal softmax attention ; out = attn @ Wo.T

Sharding (data + tensor parallel over heads):
  core c -> batch b = c // 4, heads {4g..4g+3} with g = c % 4.
  Each core computes Q,K,V for its 4 heads, runs causal attention for them,
  and multiplies by the matching 256 columns of Wo, producing a partial
  [2048, 1024] bf16 output. Host sums the 4 partials per batch in fp32.

Kernel structure (per core): baseline dense-emission skeleton with
  - bf16 matmul operands, fp32 PSUM accumulation.
  - PAIRED scores: heads 2p/2p+1 live at partition bases 0/64 of the same
    qkT m-tile, so their K^T Q matmuls (64-row stationaries) execute
    CONCURRENTLY in opposite PE row groups (row tiling), writing the two
    banks of one [128,1024] PSUM tile; one 1024-wide exp covers both.
  - AV per head unsplit ([128,65] stationary with a ones column accumulating
    the softmax denominator), emitted 2 k-blocks behind the paired scores so
    the PE tracks ACT's exp rate; normalization reads PSUM directly.
  - Causality: strictly-upper blocks skipped; diagonal straddlers compute
    only valid columns; the 128-wide diagonal block is exp'd unmasked and
    both heads are masked with one strided multiply against a stacked
    binary mask tile.
  - Input DMAs chunked [128,512] over the 3 DMA queues in consumption
    order, so the first projection chain starts after ~300KB.
"""

import os

import numpy as np

S = 2048
D = 1024
DH = 64
B = 2
NCORES = 8
HPC = 4  # heads per core
QKC = 2 * HPC * DH  # 512 q+k projection columns per core
VC = HPC * DH  # 256 v columns per core
P = 128
H = 64
KO = D // P  # 8 contraction tiles
NQ = S // 512  # 4 q-chunks of 512
NSC = S // P  # 16 s-chunks of 128

COMPUTE_DT = os.environ.get("ATTN_COMPUTE_DT", "bf16")
DEBUG = os.environ.get("ATTN_DEBUG", "0") == "1"

_cache = {}


def _np_compute_dt():
    if COMPUTE_DT == "bf16":
        import ml_dtypes

        return ml_dtypes.bfloat16
    return np.float32


def _build():
    import concourse.bacc as bacc
    import concourse.mybir as mybir
    import concourse.tile as tile

    F32 = mybir.dt.float32
    CDT = mybir.dt.bfloat16 if COMPUTE_DT == "bf16" else mybir.dt.float32r
    EXP = mybir.ActivationFunctionType.Exp

    nc = bacc.Bacc()
    xT_d = nc.dram_tensor("xT", [D, S], CDT, kind="ExternalInput")
    wqkT_d = nc.dram_tensor("wqkT", [D, QKC], CDT, kind="ExternalInput")
    wvT_d = nc.dram_tensor("wvT", [D, VC], CDT, kind="ExternalInput")
    woT_d = nc.dram_tensor("woT", [VC, D], CDT, kind="ExternalInput")
    maskT_d = nc.dram_tensor("maskT", [P, P], CDT, kind="ExternalInput")
    out_d = nc.dram_tensor("out", [S, D], CDT, kind="ExternalOutput")
    if DEBUG:
        qkT_dump = nc.dram_tensor("qkT_dump", [P, 4, S], CDT, kind="ExternalOutput")
        v_dump = nc.dram_tensor("v_dump", [P, NSC, HPC, DH + 1], CDT, kind="ExternalOutput")
        attn_dump = nc.dram_tensor("attn_dump", [P, 2, S], CDT, kind="ExternalOutput")

    with tile.TileContext(nc) as tc:
        with (
            tc.tile_pool(name="persist", bufs=1) as persist,
            tc.tile_pool(name="sb_small", bufs=3) as sb_small,
            tc.tile_pool(name="sb_exp", bufs=12) as sb_exp,
            tc.tile_pool(name="sb_out", bufs=3) as sb_out,
            tc.tile_pool(name="pp_big", bufs=2, space="PSUM") as pp_big,
            tc.tile_pool(name="pp_av", bufs=2, space="PSUM") as pp_av,
            tc.tile_pool(name="pp_o", bufs=2, space="PSUM") as pp_o,
        ):
            xT_sb = persist.tile([P, KO, S], CDT, tag="xT")
            wqkT_sb = persist.tile([P, KO, QKC], CDT, tag="wqkT")
            wvT_sb = persist.tile([P, KO, VC], CDT, tag="wvT")
            woT_sb = persist.tile([P, 2, D], CDT, tag="woT")
            maskT2_sb = persist.tile([P, 2, P], CDT, tag="maskT2")
            qkT_sb = persist.tile([P, 4, S], CDT, tag="qkT")
            v_sb = persist.tile([P, NSC, HPC, DH + 1], CDT, tag="v")
            attn_sb = persist.tile([P, 2, S], CDT, tag="attn")

            def xdma(eng, ko, qc):
                eng.dma_start(
                    xT_sb[:, ko, qc * 512 : (qc + 1) * 512],
                    xT_d[ko * P : (ko + 1) * P, qc * 512 : (qc + 1) * 512],
                )

            # Round-robin all transfers across the 3 DMA queues in strict
            # consumption order: qk-proj inputs first (wqkT[ko] + x[ko,qc0]
            # pairs), then wvT + mask, then later x chunks, then woT.
            qs = [nc.sync, nc.gpsimd]
            qi = 0

            def rr():
                nonlocal qi
                q = qs[qi % 2]
                qi += 1
                return q

            for ko in range(KO):
                rr().dma_start(wqkT_sb[:, ko, :], wqkT_d[ko * P : (ko + 1) * P, :])
                xdma(rr(), ko, 0)
            for ko in range(KO):
                rr().dma_start(wvT_sb[:, ko, :], wvT_d[ko * P : (ko + 1) * P, :])
            for _mh in range(2):
                rr().dma_start(maskT2_sb[:, _mh, :], maskT_d[:])
            for qc in range(1, NQ):
                for ko in range(KO):
                    xdma(rr(), ko, qc)
            rr().dma_start(woT_sb[:], woT_d.rearrange("(ct p) e -> p ct e", p=P))
            ones_f32 = persist.tile([P, DH], F32, tag="ones_f32")
            nc.vector.memset(ones_f32[:], 1.0)
            nc.vector.tensor_copy(
                out=v_sb[:, :, :, DH],
                in_=ones_f32[:, 0 : NSC * HPC].rearrange("p (a b) -> p a b", a=NSC),
            )

            def outproj_unit(qc, si, en, final=False):
                sc = qc * 4 + si
                ps_o = pp_o.tile([P, 512], F32, tag="o")
                for ct in range(2):
                    nc.tensor.matmul(
                        ps_o[:],
                        attn_sb[:, ct, sc * P : (sc + 1) * P],
                        woT_sb[:, ct, en * 512 : (en + 1) * 512],
                        start=(ct == 0),
                        stop=(ct == 1),
                        skip_group_check=True,
                    )
                o_sb = sb_out.tile([P, 512], CDT, tag="osb")
                if final and (si + en) % 2 == 1:
                    # drain phase: ACT is idle, use it for half the casts
                    nc.scalar.copy(out=o_sb[:], in_=ps_o[:])
                else:
                    nc.vector.tensor_copy(out=o_sb[:], in_=ps_o[:])
                if final:
                    # split across both queues to shorten the drain tail
                    nc.sync.dma_start(
                        out_d[sc * P : (sc + 1) * P, en * 512 : en * 512 + 256],
                        o_sb[:, 0:256],
                    )
                    nc.gpsimd.dma_start(
                        out_d[sc * P : (sc + 1) * P, en * 512 + 256 : (en + 1) * 512],
                        o_sb[:, 256:512],
                    )
                else:
                    (nc.sync if (si + en) % 2 == 0 else nc.gpsimd).dma_start(
                        out_d[sc * P : (sc + 1) * P, en * 512 : (en + 1) * 512],
                        o_sb[:],
                    )

            def emit_outproj(qc, final=False):
                for si in range(4):
                    for en in range(2):
                        outproj_unit(qc, si, en, final=final)

            for qc in range(NQ):
                # --- qk projection: ko-outer over two 2-bank tiles (4
                # half-bank chains) so the PE tracks DMA arrival ---
                pjA = pp_big.tile([P, 1024], F32, tag="big", name="pjA")
                pjB = pp_big.tile([P, 1024], F32, tag="big", name="pjB")
                for ko in range(KO):
                    for mm in range(4):
                        slot = pjA if mm < 2 else pjB
                        nc.tensor.matmul(
                            slot[:, (mm % 2) * 512 : (mm % 2 + 1) * 512],
                            wqkT_sb[:, ko, mm * P : (mm + 1) * P],
                            xT_sb[:, ko, qc * 512 : (qc + 1) * 512],
                            start=(ko == 0),
                            stop=(ko == KO - 1),
                            skip_group_check=True,
                        )
                nc.vector.tensor_copy(
                    out=qkT_sb[:, 0:2, qc * 512 : (qc + 1) * 512],
                    in_=pjA.rearrange("p (a b) -> p a b", a=2),
                )
                nc.vector.tensor_copy(
                    out=qkT_sb[:, 2:4, qc * 512 : (qc + 1) * 512],
                    in_=pjB.rearrange("p (a b) -> p a b", a=2),
                )

                # --- v projection for s-chunks 4qc..4qc+3 (4 bank chains) ---
                pvA = pp_big.tile([P, 1024], F32, tag="big", name="pvA")
                pvB = pp_big.tile([P, 1024], F32, tag="big", name="pvB")
                for ko in range(KO):
                    for j in range(4):
                        slot = pvA if j < 2 else pvB
                        sc = 4 * qc + j
                        nc.tensor.matmul(
                            slot[:, (j % 2) * 512 : (j % 2) * 512 + VC],
                            xT_sb[:, ko, sc * P : (sc + 1) * P],
                            wvT_sb[:, ko, :],
                            start=(ko == 0),
                            stop=(ko == KO - 1),
                            skip_group_check=True,
                        )
                for half, slot in ((0, pvA), (1, pvB)):
                    nc.vector.tensor_copy(
                        out=v_sb[:, 4 * qc + 2 * half : 4 * qc + 2 * half + 2, :, 0:DH],
                        in_=slot.rearrange("p (a h d) -> p a h d", a=2, h=8)[:, :, 0:HPC, :],
                    )

                # --- attention for q-chunk qc, head pairs (0,1), (2,3).
                # Pair heads sit at partition bases 0/64 of one qkT m-tile,
                # so the two 64-row score matmuls of a k-block execute
                # concurrently in opposite PE row groups (one 2-bank tile,
                # one 1024-wide exp). AV (unsplit, ones-column denominator)
                # trails the scores by 2 k-blocks.
                nkb = 4 * qc + 4
                for pr in range(2):
                    mq = pr  # Q m-tile; K m-tile = 2 + pr
                    ps_avs = [
                        pp_av.tile([DH + 1, 512], F32, tag="av", name=f"av{hh}")
                        for hh in range(2)
                    ]
                    exps = {}

                    def emit_scores(kb):
                        m = kb - 4 * qc
                        off = max(0, m) * P
                        ps2 = pp_big.tile([P, 1024], F32, tag="big", name="ps2")
                        exp2 = sb_exp.tile([P, 1024], CDT, tag="exp")
                        for hh in range(2):
                            hp = hh * H
                            nc.tensor.matmul(
                                ps2[:, hh * 512 + off : hh * 512 + 512],
                                qkT_sb[hp : hp + H, 2 + mq, kb * P : (kb + 1) * P],
                                qkT_sb[hp : hp + H, mq, qc * 512 + off : (qc + 1) * 512],
                                start=True,
                                stop=True,
                                skip_group_check=True,
                            )
                        if off == 0:
                            nc.scalar.activation(exp2[:], ps2[:], EXP, scale=0.125)
                        else:
                            for hh in range(2):
                                lo = hh * 512 + off
                                nc.scalar.activation(
                                    exp2[:, lo : hh * 512 + 512],
                                    ps2[:, lo : hh * 512 + 512],
                                    EXP,
                                    scale=0.125,
                                )
                        if m >= 0:
                            e2v = exp2.rearrange("p (h q) -> p h q", h=2)[
                                :, :, off : off + P
                            ]
                            nc.vector.tensor_mul(out=e2v, in0=e2v, in1=maskT2_sb[:])
                        exps[kb] = (exp2, off)

                    def emit_av(kb):
                        exp2, off = exps[kb]
                        for hh in range(2):
                            h = 2 * pr + hh
                            lo = hh * 512 + off
                            nc.tensor.matmul(
                                ps_avs[hh][:, off:512],
                                v_sb[:, kb, h, :],
                                exp2[:, lo : (lo - off) + 512],
                                start=(kb == 0),
                                stop=(kb == nkb - 1),
                                skip_group_check=True,
                            )

                    for kb in range(nkb):
                        emit_scores(kb)
                        if kb >= 2:
                            emit_av(kb - 2)
                    emit_av(nkb - 2)
                    emit_av(nkb - 1)

                    # normalize: attn = av * (1/sums), reciprocal broadcast
                    # over the 64 head dims via GPSIMD (keeps the PE out of
                    # the pair-boundary dependency chain).
                    sums2 = sb_small.tile([1, 1024], F32, tag="sums2")
                    for hh in range(2):
                        nc.vector.tensor_copy(
                            out=sums2[:, hh * 512 : (hh + 1) * 512],
                            in_=ps_avs[hh][DH : DH + 1, :],
                        )
                    recip2 = sb_small.tile([1, 1024], F32, tag="recip2")
                    nc.vector.reciprocal_approx_fast(out=recip2[:], in_=sums2[:])
                    for hh in range(2):
                        bc_sb = sb_small.tile([DH, 512], F32, tag=f"bc{hh}")
                        nc.gpsimd.partition_broadcast(
                            bc_sb[:], recip2[:, hh * 512 : (hh + 1) * 512]
                        )
                        hp = hh * H
                        nc.vector.tensor_mul(
                            out=attn_sb[hp : hp + DH, pr, qc * 512 : (qc + 1) * 512],
                            in0=ps_avs[hh][0:DH, :],
                            in1=bc_sb[:],
                        )

                # --- deferred output projection (previous q chunk) ---
                if qc > 0:
                    emit_outproj(qc - 1)
            emit_outproj(NQ - 1, final=True)
            if DEBUG:
                nc.sync.dma_start(qkT_dump[:], qkT_sb[:])
                nc.sync.dma_start(v_dump[:], v_sb[:])
                nc.sync.dma_start(attn_dump[:], attn_sb[:])

    nc.compile()
    return nc


def _get_nc():
    if "nc" not in _cache:
        _cache["nc"] = _build()
    return _cache["nc"]


def _shard(x, mask, Wqkv, Wo):
    cdt = _np_compute_dt()
    in_maps = []
    # binary mask for the transposed 128x128 diagonal block:
    # valid (mask==0) -> 1.0, masked (-inf/large-negative) -> 0.0
    maskT = np.ascontiguousarray((mask[0, 0, :P, :P].T >= 0).astype(cdt))
    for c in range(NCORES):
        b = c // 4
        g = c % 4
        heads = [4 * g + i for i in range(HPC)]
        q_rows = np.concatenate([np.arange(h * DH, (h + 1) * DH) for h in heads])
        k_rows = D + q_rows
        v_rows = 2 * D + q_rows
        qk_rows = np.concatenate([q_rows, k_rows])
        in_maps.append(
            {
                "xT": np.ascontiguousarray(x[b].T.astype(cdt)),
                "wqkT": np.ascontiguousarray(Wqkv[qk_rows, :].T.astype(cdt)),
                "wvT": np.ascontiguousarray(Wqkv[v_rows, :].T.astype(cdt)),
                "woT": np.ascontiguousarray(Wo[:, q_rows].T.astype(cdt)),
                "maskT": maskT,
            }
        )
    return in_maps


def kernel(x, mask, Wqkv, Wo, _trace=False):
    from concourse.bass_utils import run_bass_kernel_spmd

    x = np.asarray(x, dtype=np.float32)
    mask = np.asarray(mask, dtype=np.float32)
    Wqkv = np.asarray(Wqkv, dtype=np.float32)
    Wo = np.asarray(Wo, dtype=np.float32)

    nc = _get_nc()
    in_maps = _shard(x, mask, Wqkv, Wo)
    res = run_bass_kernel_spmd(nc, in_maps, core_ids=list(range(NCORES)), trace=_trace)
    _cache["last_result"] = res

    out = np.zeros((B, S, D), dtype=np.float32)
    for c in range(NCORES):
        out[c // 4] += res.results[c]["out"].astype(np.float32)
    return out
